# revision 49
# baseline (speedup 1.0000x reference)
"""Multi-head attention Trainium2 kernel (B=4, S=2048, E=1024, H=16).

Sharding: 8 cores = 4 batch groups x 2-way head tensor-parallel.
Core c handles batch b=c//2 and heads [g*8, g*8+8) with g=c%2.
Partial output projections are pair-summed with a 4-chunk bf16
ReduceScatter whose outputs are kernel outputs; the host assembles
and casts to f32 (identical values to an on-chip cast).

v5 design notes:
- All matmuls stay in plain 128x128 PE mode (Q/K zero-padded to 128
  contraction rows).  v4 tried 64x128 row-tiled scores: the pairs DO
  run concurrently (4ns stagger), but the PE drains at every
  64-mode<->128-mode switch, so each matmul ran isolated (~410ns for
  512 cols vs ~213 back-to-back) and the net was a 20us regression.
- Strips iterate (qph, hp, kb): qph = 512-row query block, hp = head
  pair.  One [128, 1024] exp per strip (even|odd head halves).
- qph-major order finishes output rows in 512-row quarters, so the
  output projection and the pairwise bf16 ReduceScatter chunks fire
  throughout the run; only the last 512-row chunk's latency is a
  serial tail.  RS enqueues (gpsimd) are deferred to stint starts so
  their data-ready waits never stall the normalize broadcasts.
- P@V results spill PSUM->SBUF immediately (raw, with the ones-column
  denominator); normalization (gpsimd partition_broadcast + DVE
  reciprocal/multiply) is deferred off the critical path.
- V carries a ones-column so P@V also emits the softmax denominator.
- Input DMAs go demand-ordered over 4 hwdge queues; wq/wk arrive in
  per-head-pair column slices.  A dummy exp warms the ACT table RAM
  during the input DMA.
"""

import os
import sys

import numpy as np

for _p in ("/opt/trn_rl_repo", "/root/.axon_site/_ro/trn_rl_repo"):
    if os.path.isdir(_p) and _p not in sys.path:
        sys.path.append(_p)

import ml_dtypes  # noqa: E402
from concourse import bacc, mybir, tile  # noqa: E402
from concourse.bass_utils import run_bass_kernel_spmd  # noqa: E402

B, S, E, H, DH = 4, 2048, 1024, 16, 64
N_CORES = 8
TP = 2  # head-parallel factor within a batch
H_LOC = H // TP  # 8 heads per core
EI_LOC = H_LOC * DH  # 512 local rows of the concat dim
N_SB = S // 128  # 16 token blocks
N_EC = E // 128  # 8 contraction chunks
N_QB = S // 512  # 4 query blocks (= qph blocks)
N_KB = S // 128  # 16 key blocks
N_HP = H_LOC // 2  # 4 head pairs
# ReduceScatter chunks: (row0, nrows); chunk i covers output rows
# [row0, row0+nrows) and fires once its out-proj stores have landed.
RS_CHUNKS = [(0, 512), (512, 512), (1024, 512), (1536, 512)]

BF = mybir.dt.bfloat16
F32 = mybir.dt.float32
EXP = mybir.ActivationFunctionType.Exp
MULT = mybir.AluOpType.mult

_CACHE = {}


def _build():
    nc = bacc.Bacc("TRN2", target_bir_lowering=False, debug=False,
                   num_devices=N_CORES)

    xT_in = nc.declare_dram_parameter("xT", [E, S], BF, isOutput=False)
    wq_in = nc.declare_dram_parameter("wq", [E, EI_LOC], BF, isOutput=False)
    wk_in = nc.declare_dram_parameter("wk", [E, EI_LOC], BF, isOutput=False)
    wv_in = nc.declare_dram_parameter("wv", [E, EI_LOC], BF, isOutput=False)
    woT_in = nc.declare_dram_parameter("woT", [EI_LOC, E], BF, isOutput=False)
    bob_in = nc.declare_dram_parameter("bob", [128, E], F32, isOutput=False)
    zpad_in = nc.declare_dram_parameter("zpad", [64, S], BF, isOutput=False)
    y_out = nc.declare_dram_parameter("y", [S // TP, E], BF, isOutput=True)

    y_part = nc.dram_tensor("y_part", [S, E], BF)
    y_chunks = [nc.dram_tensor(f"y_chunk{i}", [n // 2, E], BF)
                for i, (_, n) in enumerate(RS_CHUNKS)]

    inv_sqrt_dh = 1.0 / float(np.sqrt(DH))

    with tile.TileContext(nc) as tc:
        with (
            tc.tile_pool(name="const", bufs=1) as constp,
            tc.tile_pool(name="persist", bufs=1) as persist,
            tc.tile_pool(name="scps", bufs=2, space="PSUM") as scps,
            tc.tile_pool(name="pvps", bufs=2, space="PSUM") as pvps,
            tc.tile_pool(name="mixps", bufs=1, space="PSUM") as mixps,
            tc.tile_pool(name="ptp", bufs=5) as ptp,
            tc.tile_pool(name="pvsb", bufs=6) as pvsb,
            tc.tile_pool(name="smalldn", bufs=2) as smalldn,
            tc.tile_pool(name="denbp", bufs=4) as denbp,
            tc.tile_pool(name="youtp", bufs=2) as youtp,
        ):
            xTp = tc.alloc_tile_pool(name="xTp", bufs=1)
            xT = [xTp.tile([128, S], BF, tag=f"xT{ec}", name=f"xT{ec}")
                  for ec in range(N_EC)]

            # ---- ACT table warm-up: pay the exp table load during DMA ----
            warm_in = constp.tile([128, 8], F32, tag="warm", name="warm_in")
            nc.vector.memset(warm_in[:], 0.0)
            warm_out = constp.tile([128, 8], BF, tag="wout", name="warm_out")
            nc.scalar.activation(warm_out[:], warm_in[:], EXP)

            # ---- input DMAs: demand order ----
            # A dma_start costs ~0.6us of SWDGE descriptor time on the
            # issuing queue, so: phase A uses all 4 free queues, later
            # phases only sync+gpsimd (scalar must clear before the exp
            # stream starts, vector before the first V copies).
            qst = dict(i=0)

            def dma(dst, src, queues):
                queues[qst["i"] % len(queues)].dma_start(dst, src)
                qst["i"] += 1

            qall = (nc.sync, nc.scalar, nc.gpsimd)
            qsg = (nc.sync, nc.gpsimd)

            wv_t = [constp.tile([128, EI_LOC], BF, tag=f"wv{ec}",
                                name=f"wv{ec}") for ec in range(N_EC)]
            wq_t = [constp.tile([128, EI_LOC], BF, tag=f"wq{ec}",
                                name=f"wq{ec}") for ec in range(N_EC)]
            wk_t = [constp.tile([128, EI_LOC], BF, tag=f"wk{ec}",
                                name=f"wk{ec}") for ec in range(N_EC)]
            # Per-head Q/K, zero-padded rows 64-127 so all matmuls stay in
            # plain 128x128 mode (v4's 64x128 row-tiled scores ran
            # concurrently but the per-strip mode-switch drain cost more
            # than the concurrency won).  Pads come from a host-side zeros
            # input: DVE memsets here measured 1.76us each and stalled the
            # first QK copy (and first exp) behind ~34us of memsets.
            QT = [persist.tile([128, S], BF, tag=f"QT{h}", name=f"QT{h}")
                  for h in range(H_LOC)]
            KT = [persist.tile([128, S], BF, tag=f"KT{h}", name=f"KT{h}")
                  for h in range(H_LOC)]

            # A) first-scores gate: x cols 0-511, q/k pair-0 weight slices,
            #    pair-0 Q/K zero-pads; then V weights (v0/v1 follow)
            for ec in range(N_EC):
                dma(xT[ec][:, 0:512], xT_in[ec * 128:(ec + 1) * 128, 0:512],
                    qall)
                dma(wq_t[ec][:, 0:128], wq_in[ec * 128:(ec + 1) * 128, 0:128],
                    qall)
                dma(wk_t[ec][:, 0:128], wk_in[ec * 128:(ec + 1) * 128, 0:128],
                    qall)
            for h in (0, 1):
                dma(QT[h][64:128, :], zpad_in[:], qall)
                dma(KT[h][64:128, :], zpad_in[:], qall)
            for ec in range(N_EC):
                dma(wv_t[ec][:], wv_in[ec * 128:(ec + 1) * 128, :], qall)
            # B) x cols 512-1023 (v4-v7 + qb1 projections)
            for ec in range(N_EC):
                dma(xT[ec][:, 512:1024],
                    xT_in[ec * 128:(ec + 1) * 128, 512:1024], qsg)
            for h in (2, 3):
                dma(QT[h][64:128, :], zpad_in[:], qsg)
                dma(KT[h][64:128, :], zpad_in[:], qsg)
            # C) rest, demand-ordered: pair-1 weights, x tail (v8-v15),
            #    remaining pads, pair-2/3 weights, out-proj weights + bias
            for cs in (slice(128, 256),):
                for w_in, w_t in ((wk_in, wk_t), (wq_in, wq_t)):
                    for ec in range(N_EC):
                        dma(w_t[ec][:, cs], w_in[ec * 128:(ec + 1) * 128, cs],
                            qsg)
            for cb in (2, 3):
                cs = slice(cb * 512, (cb + 1) * 512)
                for ec in range(N_EC):
                    dma(xT[ec][:, cs], xT_in[ec * 128:(ec + 1) * 128, cs],
                        qsg)
            for h in (4, 5, 6, 7):
                dma(QT[h][64:128, :], zpad_in[:], qsg)
                dma(KT[h][64:128, :], zpad_in[:], qsg)
            for hp in (2, 3):
                cs = slice(hp * 128, (hp + 1) * 128)
                for w_in, w_t in ((wk_in, wk_t), (wq_in, wq_t)):
                    for ec in range(N_EC):
                        dma(w_t[ec][:, cs], w_in[ec * 128:(ec + 1) * 128, cs],
                            qsg)
            woT_t = []
            for c in range(4):
                t = constp.tile([128, E], BF, tag=f"woT{c}", name=f"woT{c}")
                dma(t[:], woT_in[c * 128:(c + 1) * 128, :], qsg)
                woT_t.append(t)
            bob = constp.tile([128, E], F32, tag="bob")
            dma(bob[:], bob_in[:], qsg)

            # ---- persistent SBUF tiles (QT/KT declared above w/ DMAs) ----
            V = [persist.tile([128, H_LOC, DH + 1], BF, tag=f"V{s}",
                              name=f"V{s}") for s in range(N_SB)]
            # CT[qp][c]: concat rows [c*128, (c+1)*128) x query rows
            # [qp*1024, (qp+1)*1024); chunk c holds heads 2c, 2c+1.
            CT = [[persist.tile([128, S // 2], BF, tag=f"CT{qp}_{c}",
                                name=f"CT{qp}_{c}") for c in range(4)]
                  for qp in range(2)]

            st = dict(rs_fired=0)

            # ---- deferred PE work units (woven into attention slack) ----
            def v_unit(sb, pool, tag):
                ps = pool.tile([128, 1024], F32, tag=tag, name="vps")
                for ec in range(N_EC):
                    nc.tensor.matmul(
                        ps[:, 0:EI_LOC], xT[ec][:, sb * 128:(sb + 1) * 128],
                        wv_t[ec][:], start=(ec == 0), stop=(ec == N_EC - 1))
                nc.vector.tensor_copy(V[sb][:, :, 0:DH], ps[:, 0:EI_LOC])
                nc.vector.memset(V[sb][:, :, DH], 1.0)

            def qk_unit(hp, qb, which, pool, tag):
                w = wq_t if which == "q" else wk_t
                dst = QT if which == "q" else KT
                ps = pool.tile([128, 1024], F32, tag=tag, name="qkps")
                for ec in range(N_EC):
                    nc.tensor.matmul(
                        ps[:, 0:512],
                        w[ec][:, hp * 128:(hp + 1) * 128],
                        xT[ec][:, qb * 512:(qb + 1) * 512],
                        start=(ec == 0), stop=(ec == N_EC - 1))
                cols = slice(qb * 512, (qb + 1) * 512)
                for hh in range(2):
                    h = 2 * hp + hh
                    rows = slice(hh * 64, (hh + 1) * 64)
                    nc.vector.tensor_copy(dst[h][0:64, cols],
                                          ps[rows, 0:512])

            def outproj_unit(sb, pool, tag, q=None):
                ct = CT[sb // 8]
                cs = slice((sb % 8) * 128, (sb % 8 + 1) * 128)
                ys = pool.tile([128, E], F32, tag=tag, name="ys")
                for eo in range(2):
                    for c in range(4):
                        nc.tensor.matmul(
                            ys[:, eo * 512:(eo + 1) * 512],
                            ct[c][:, cs],
                            woT_t[c][:, eo * 512:(eo + 1) * 512],
                            start=(c == 0), stop=(c == 3))
                yt = youtp.tile([128, E], BF, tag="yt", name="yt")
                nc.vector.tensor_add(yt[:], ys[:], bob[:])
                (q or nc.sync).dma_start(
                    y_part[sb * 128:(sb + 1) * 128, :], yt[:])

            def fire_rs(i):
                r0, n = RS_CHUNKS[i]
                nc.gpsimd.collective_compute(
                    "ReduceScatter", mybir.AluOpType.add,
                    replica_groups=[[0, 1], [2, 3], [4, 5], [6, 7]],
                    ins=[y_part[r0:r0 + n, :]],
                    outs=[y_chunks[i][:]])
                st["rs_fired"] += 1

            # ---- stepped filler units ----
            # Each deferred unit is ~1.7us of PE work; emitted whole it
            # delays the next scores matmul past the exp period and the
            # ACT stream gaps (measured as paired ~1.5us gaps at every
            # weave point).  As generators they emit 2-matmul (~0.43us)
            # chunks per run_filler() call, fitting the per-strip slack.
            def v_mini(sb, p, pool, tag):
                # V projection for ONE head pair (N=128): stint 0 only
                # consumes heads 0-1, so producing V in pair slices cuts
                # its unit demand from 27us to ~10us; pairs 1-3 fill the
                # slack of later stints.  ~0.45us each - no stepping.
                ps = pool.tile([128, 1024], F32, tag=tag, name="vps")
                for ec in range(N_EC):
                    nc.tensor.matmul(
                        ps[:, 0:128],
                        xT[ec][:, sb * 128:(sb + 1) * 128],
                        wv_t[ec][:, p * 128:(p + 1) * 128],
                        start=(ec == 0), stop=(ec == N_EC - 1))
                nc.vector.tensor_copy(V[sb][:, 2 * p:2 * p + 2, 0:DH],
                                      ps[:, 0:128])
                if p == 0:
                    # ones column for ALL heads (independent of V data)
                    nc.vector.memset(V[sb][:, :, DH], 1.0)

            def qk_unit_gen(hp, qb, which, pool, tag):
                w = wq_t if which == "q" else wk_t
                dst = QT if which == "q" else KT
                ps = pool.tile([128, 1024], F32, tag=tag, name="qkps")
                for step in range(4):
                    for ec in (2 * step, 2 * step + 1):
                        nc.tensor.matmul(
                            ps[:, 0:512],
                            w[ec][:, hp * 128:(hp + 1) * 128],
                            xT[ec][:, qb * 512:(qb + 1) * 512],
                            start=(ec == 0), stop=(ec == N_EC - 1))
                    if step < 3:
                        yield
                cols = slice(qb * 512, (qb + 1) * 512)
                for hh in range(2):
                    h = 2 * hp + hh
                    rows = slice(hh * 64, (hh + 1) * 64)
                    nc.vector.tensor_copy(dst[h][0:64, cols],
                                          ps[rows, 0:512])

            def outproj_unit_gen(sb, pool, tag):
                ct = CT[sb // 8]
                cs = slice((sb % 8) * 128, (sb % 8 + 1) * 128)
                ys = pool.tile([128, E], F32, tag=tag, name="ys")
                for eo in range(2):
                    for cp in range(2):
                        for c in (2 * cp, 2 * cp + 1):
                            nc.tensor.matmul(
                                ys[:, eo * 512:(eo + 1) * 512],
                                ct[c][:, cs],
                                woT_t[c][:, eo * 512:(eo + 1) * 512],
                                start=(c == 0), stop=(c == 3))
                        if not (eo == 1 and cp == 1):
                            yield
                yt = youtp.tile([128, E], BF, tag="yt", name="yt")
                nc.vector.tensor_add(yt[:], ys[:], bob[:])
                nc.sync.dma_start(y_part[sb * 128:(sb + 1) * 128, :], yt[:])

            # ---- prologue: just enough for strip (qph0, hp0, kb0) ----
            # q/k alone gate the first scores + exp; V[0]/V[1] are only
            # needed by the (2-lagged) first P@V, so they run as the
            # first (atomic) fillers during strips 0-1.  The two chains
            # interleave per-ec to match DMA chunk arrival (phase A loads
            # xT/wq/wk per ec together), so both finish with the last
            # chunk instead of serially.
            psq = scps.tile([128, 1024], F32, tag="sc", name="psq")
            psk = scps.tile([128, 1024], F32, tag="sc", name="psk")
            for ec in range(N_EC):
                nc.tensor.matmul(psq[:, 0:512], wq_t[ec][:, 0:128],
                                 xT[ec][:, 0:512],
                                 start=(ec == 0), stop=(ec == N_EC - 1))
                nc.tensor.matmul(psk[:, 0:512], wk_t[ec][:, 0:128],
                                 xT[ec][:, 0:512],
                                 start=(ec == 0), stop=(ec == N_EC - 1))
            for hh in range(2):
                rows = slice(hh * 64, (hh + 1) * 64)
                nc.vector.tensor_copy(QT[hh][0:64, 0:512],
                                      psq[rows, 0:512])
                nc.vector.tensor_copy(KT[hh][0:64, 0:512],
                                      psk[rows, 0:512])

            # deferred work queue, demand-ordered for the interleaved
            # stint schedule below: V[kb] is needed by strip kb+2; pair
            # p's KT/QT-qb0 by stint 2p (strip 32p); QT-qb1 by stint
            # 2p+1; later Q blocks are staged per-stint.
            filler = [
                ("v", 0, 0), ("v", 1, 0),
                ("v", 2, 0), ("qk", 0, 1, "k"), ("v", 3, 0), ("v", 4, 0),
                ("v", 5, 0), ("qk", 0, 2, "k"), ("v", 6, 0), ("v", 7, 0),
                ("qk", 0, 1, "q"),
                ("v", 8, 0), ("qk", 0, 3, "k"), ("v", 9, 0), ("v", 10, 0),
                ("v", 11, 0), ("v", 12, 0),
                ("v", 13, 0), ("v", 14, 0), ("v", 15, 0),
            ]
            # pairs 1-3: each pair p's K columns + V slices are demanded
            # from stint 2p (strip 32p) onward, interleaved by deadline
            for p in (1, 2, 3):
                filler += [("qk", p, 0, "k"), ("qk", p, 0, "q"),
                           ("v", 0, p), ("v", 1, p), ("v", 2, p),
                           ("qk", p, 1, "k"),
                           ("v", 3, p), ("v", 4, p), ("v", 5, p),
                           ("qk", p, 2, "k"),
                           ("v", 6, p), ("v", 7, p), ("v", 8, p),
                           ("qk", p, 3, "k"),
                           ("v", 9, p), ("v", 10, p), ("v", 11, p),
                           ("v", 12, p), ("v", 13, p), ("v", 14, p),
                           ("v", 15, p),
                           ("qk", p, 1, "q")]
            st["gen"] = None

            def run_filler(n=1):
                for _ in range(n):
                    if st["gen"] is None:
                        if not filler:
                            return
                        kind = filler.pop(0)
                        if kind[0] == "v":
                            v_mini(kind[1], kind[2], mixps, "mx")
                            continue
                        if kind[0] == "qk":
                            _, hp, qb, which = kind
                            st["gen"] = qk_unit_gen(hp, qb, which,
                                                    mixps, "mx")
                        else:
                            st["gen"] = outproj_unit_gen(kind[1], mixps,
                                                         "mx")
                    try:
                        next(st["gen"])
                    except StopIteration:
                        st["gen"] = None

            # ---- attention stream ----
            # Strip g = (qph, hp, kb): scores for both heads of pair hp at
            # key block kb, query rows [qph*512, (qph+1)*512).
            # qph0/qph1 stints interleave so the KT/QT unit demand of the
            # first query block spreads over twice the window; qph2/qph3
            # stay sequential so only the final RS chunk is a serial tail.
            stints = [(0, 0), (1, 0), (0, 1), (1, 1),
                      (0, 2), (1, 2), (0, 3), (1, 3),
                      (2, 0), (2, 1), (2, 2), (2, 3),
                      (3, 0), (3, 1), (3, 2), (3, 3)]
            strips = [(qph, hp, kb)
                      for (qph, hp) in stints
                      for kb in range(N_KB)]
            n_strips = len(strips)
            PV_LAG = 2  # P@V(g) issues with strip g+2: exp(g) has drained
            pts = {}       # strip idx -> PT tile
            pvt = {}       # (qph, hp) -> (pv_even, pv_odd)
            norm_q = []    # deferred normalize ops

            def do_scores(g):
                qph, hp, kb = strips[g]
                qs = slice(qph * 512, (qph + 1) * 512)
                ks = slice(kb * 128, (kb + 1) * 128)
                sp = scps.tile([128, 1024], F32, tag="sc", name="sc")
                # even head -> cols 0-511, odd head -> cols 512-1023; one
                # [128, 1024] exp covers the pair.
                nc.tensor.matmul(sp[:, 0:512], KT[2 * hp][:, ks],
                                 QT[2 * hp][:, qs])
                nc.tensor.matmul(sp[:, 512:1024], KT[2 * hp + 1][:, ks],
                                 QT[2 * hp + 1][:, qs])
                pt = ptp.tile([128, 1024], BF, tag="pt", name="pt")
                nc.scalar.activation(pt[:], sp[:], EXP, scale=inv_sqrt_dh)
                pts[g] = pt

            def do_pv(g):
                qph, hp, kb = strips[g]
                if kb == 0:
                    pvt[(qph, hp)] = (
                        pvps.tile([DH + 1, 512], F32, tag="pv", name="pve"),
                        pvps.tile([DH + 1, 512], F32, tag="pv", name="pvo"))
                pve, pvo = pvt[(qph, hp)]
                pt = pts.pop(g)
                nc.tensor.matmul(pve[:], V[kb][:, 2 * hp, :], pt[:, 0:512],
                                 start=(kb == 0), stop=(kb == N_KB - 1))
                nc.tensor.matmul(pvo[:], V[kb][:, 2 * hp + 1, :],
                                 pt[:, 512:1024],
                                 start=(kb == 0), stop=(kb == N_KB - 1))
                if kb == N_KB - 1:
                    # spill raw P@V to SBUF so the psum pair recycles
                    # immediately; normalize later.
                    for hh, pv in ((0, pve), (1, pvo)):
                        sb_t = pvsb.tile([DH, 512], F32, tag="pvs",
                                         name="pvs")
                        nc.vector.tensor_copy(sb_t[:], pv[0:DH, :])
                        den = smalldn.tile([1, 512], F32, tag="den",
                                           name="den")
                        nc.vector.tensor_copy(den[:], pv[DH:DH + 1, :])
                        denb = denbp.tile([64, 512], F32, tag="denb",
                                          name="denb")
                        nc.gpsimd.partition_broadcast(denb[:], den[:])
                        norm_q.append((qph, hp, hh, sb_t, denb))

            def flush_norms(keep=0):
                while len(norm_q) > keep:
                    qph, hp, hh, sb_t, denb = norm_q.pop(0)
                    rows = slice(hh * 64, (hh + 1) * 64)
                    qs = slice((qph % 2) * 512, (qph % 2 + 1) * 512)
                    rec = smalldn.tile([64, 512], F32, tag="rec",
                                       name="rec")
                    nc.vector.reciprocal_approx_fast(rec[:], denb[:])
                    nc.vector.tensor_tensor(
                        CT[qph // 2][hp][rows, qs], sb_t[:], rec[:], MULT)

            # per-stint-start actions: (full_flush, op_sbs, rs_chunk,
            # q_units_to_stage)
            stint_actions = {
                4: (False, (), None, [("qk", 0, 2, "q")]),
                5: (False, (), None, [("qk", 1, 2, "q")]),
                6: (False, (), None, [("qk", 2, 2, "q")]),
                7: (False, (), None, [("qk", 3, 2, "q")]),
                8: (False, (), None, [("qk", 0, 3, "q")]),
                9: (True, tuple(range(0, 8)), None, [("qk", 1, 3, "q")]),
                10: (False, (), 0, [("qk", 2, 3, "q")]),
                11: (False, (), 1, [("qk", 3, 3, "q")]),
                13: (True, tuple(range(8, 12)), None, []),
                14: (False, (), 2, []),
            }

            for g in range(n_strips + PV_LAG):
                if g < n_strips:
                    qph, hp, kb = strips[g]
                    if kb == 0:
                        si = g // N_KB
                        full, ops, rs, qs_units = stint_actions.get(
                            si, (False, (), None, []))
                        flush_norms(keep=0 if full else 2)
                        filler.extend(("op", sb) for sb in ops)
                        filler.extend(qs_units)
                        if rs is not None:
                            fire_rs(rs)
                    if g == n_strips - 8:
                        # mid-way through the last stint: all norms except
                        # the final pair are spilled and broadcast-aged;
                        # flushing now takes them off the tail's critical
                        # path into the final out-proj + ReduceScatter.
                        flush_norms(keep=0)
                    do_scores(g)
                    run_filler(1)
                if g >= PV_LAG:
                    do_pv(g - PV_LAG)
                if g < n_strips:
                    run_filler(2)

            # ---- tail: last quarter out-proj + final RS chunk ----
            flush_norms()
            while filler or st["gen"] is not None:
                run_filler(8)
            # the exp stream is done - the scalar queue is free, so the
            # four final stores alternate sync/scalar instead of queuing
            # up behind each other (the last store gates the final RS)
            for sb in range(12, 16):
                outproj_unit(sb, mixps if sb % 2 == 0 else scps,
                             "mx" if sb % 2 == 0 else "sc",
                             q=nc.scalar if sb % 2 == 0 else nc.sync)
            fire_rs(3)
            # Collectives may not write IO tensors; copy each finished
            # chunk DRAM->DRAM into the output (chunks 0-2 completed long
            # ago, only chunk 3's ~0.5MB copy is an exposed tail).
            for i, (r0, n) in enumerate(RS_CHUNKS):
                half = n // 2
                nc.sync.dma_start(y_out[r0 // 2:r0 // 2 + half, :],
                                  y_chunks[i][:])
            xTp.release()

    nc.finalize()
    return nc


def _get_nc():
    if "nc" not in _CACHE:
        _CACHE["nc"] = _build()
    return _CACHE["nc"]


def _make_in_maps(x, wq, wk, wv, wo, bo):
    bf16 = ml_dtypes.bfloat16
    x, wq, wk, wv, wo, bo = (np.asarray(a) for a in (x, wq, wk, wv, wo, bo))
    in_maps = []
    for c in range(N_CORES):
        b, g = c // TP, c % TP
        h0 = g * H_LOC
        xT_l = np.ascontiguousarray(x[b].T).astype(bf16)
        wq_l = np.ascontiguousarray(
            wq[h0:h0 + H_LOC].transpose(1, 0, 2).reshape(E, EI_LOC)).astype(bf16)
        wk_l = np.ascontiguousarray(
            wk[h0:h0 + H_LOC].transpose(1, 0, 2).reshape(E, EI_LOC)).astype(bf16)
        wv_l = np.ascontiguousarray(
            wv[h0:h0 + H_LOC].transpose(1, 0, 2).reshape(E, EI_LOC)).astype(bf16)
        woT_l = np.ascontiguousarray(
            wo[:, g * EI_LOC:(g + 1) * EI_LOC].T).astype(bf16)
        bob = np.broadcast_to(bo.astype(np.float32) / TP, (128, E)).copy()
        in_maps.append({
            "xT": xT_l, "wq": wq_l, "wk": wk_l, "wv": wv_l, "woT": woT_l,
            "bob": bob, "zpad": np.zeros((64, S), dtype=bf16),
        })
    return in_maps


def _assemble(results):
    out = np.empty((B, S, E), dtype=np.float32)
    for c in range(N_CORES):
        b, g = c // TP, c % TP
        y = results[c]["y"].astype(np.float32)
        for i, (r0, n) in enumerate(RS_CHUNKS):
            half = n // 2
            out[b, r0 + g * half:r0 + (g + 1) * half, :] = \
                y[r0 // 2:r0 // 2 + half, :]
    return out


def kernel(x, wq, wk, wv, wo, bo):
    nc = _get_nc()
    in_maps = _make_in_maps(x, wq, wk, wv, wo, bo)
    res = run_bass_kernel_spmd(nc, in_maps, list(range(N_CORES)))
    return _assemble(res.results)
